# revision 57
# baseline (speedup 1.0000x reference)
"""MLA (multi-head latent attention) Bass kernel for Trainium2, 8 NeuronCores.

Sharding: core i handles batch b = i // 2 and head-group g = i % 2
(8 of the 16 heads).  Each core computes a partial output
(its heads' contribution through out_proj, plus b_o/2); the host sums
the two partials per batch.

v2 design notes (all driven by the CoreSim v1 cost model):
  - The host pre-transposes x to [dim, S] and pre-casts x + all weights
    to bf16, so there are no on-chip PE transposes and no weight staging
    copies.  bf16 matmuls are 1 cycle/row at any moving width (f32r
    degrades 4x below 256), and halve SBUF/DMA traffic.
  - Row-vector biases (b_kvu_v, b_o) arrive pre-broadcast as [128, n]
    tiles and are folded into the PSUM->SBUF drain via tensor_add on the
    DVE, so the PE never runs bias matmuls.  Per-partition biases
    (b_kvc/b_qc/b_qu/b_kvu_k) fold into tensor_scalar_add drains.
  - Layout: everything "t-major" (feature dim on partitions):
      xT [8x128, S] (DMA'd), kv_latT [128,S], q_latT [256,S],
      KT/QT [128 = 2 heads x 64, 4 chunks x S] bf16,
      V [128 tokens, 16 chunks x (8 heads x 65)] bf16 with a ones
      column per head (col 64) that accumulates the softmax denominator
      during the PV matmul (row 64 of ctx PSUM).
  - Attention per (s-half j, head h): stream key chunks k; QK scores to
    PSUM, exp(s/8) on the scalar engine (the only Activation-engine
    work in the kernel), causal diagonal via gpsimd affine_select, PV
    accumulates ctx^T[65, SH] in PSUM; normalize via DVE reciprocal +
    gpsimd partition_broadcast + DVE multiply into ctxT bf16.
  - Software pipelining by emission order: a minimal projection prefix,
    then attention j=0 interleaved with the remaining projection work,
    then attention j=1 interleaved with out_proj of the first half,
    then the out_proj tail.  Interleaved "fill" work keeps the PE busy
    while the scalar engine (the attention-phase bottleneck) runs exp.
  - PSUM budget (8 banks): sc slots 2x[128,1024]f32 = 4, ctx
    [65,1024]f32 = 2, shared misc pool [128,512]f32 x2 = 2.
"""

import numpy as np

import concourse.bass as bass
import concourse.bacc as bacc
import concourse.mybir as mybir
import concourse.tile as tile

DIM = 1024
NUM_HEADS = 16
HEAD_DIM = 64
LAT = 128
QR = 256
B = 4
NCORES = 8
ND = DIM // 128       # 8 d-chunks
NHL = 8               # heads per core
F32 = mybir.dt.float32
BF16 = mybir.dt.bfloat16
AF = mybir.ActivationFunctionType


def _pieces(total, w=512):
    return [(o, min(w, total - o)) for o in range(0, total, w)]


def build_mla(S=2048):
    """Build the per-core Bass program (same SPMD program on all 8 cores)."""
    assert S % 1024 == 0
    SH = S // 2           # s-half width
    NT = S // 128         # number of 128-token chunks
    NP = S // 512         # number of 512-token projection pieces

    nc = bacc.Bacc()

    xT_d = nc.declare_dram_parameter("xT", [DIM, S], BF16, isOutput=False)
    w_kvc_d = nc.declare_dram_parameter("w_kvc", [DIM, LAT], BF16, isOutput=False)
    w_qc_d = nc.declare_dram_parameter("w_qc", [DIM, QR], BF16, isOutput=False)
    w_kvu_k_d = nc.declare_dram_parameter("w_kvu_k", [LAT, 512], BF16, isOutput=False)
    w_kvu_v_d = nc.declare_dram_parameter("w_kvu_v", [LAT, 512], BF16, isOutput=False)
    w_qu_d = nc.declare_dram_parameter("w_qu", [QR, 512], BF16, isOutput=False)
    w_o_d = nc.declare_dram_parameter("w_o", [512, DIM], BF16, isOutput=False)
    b_kvc_d = nc.declare_dram_parameter("b_kvc", [LAT, 1], F32, isOutput=False)
    b_qc_d = nc.declare_dram_parameter("b_qc", [128, 2], F32, isOutput=False)
    b_qu_d = nc.declare_dram_parameter("b_qu", [128, 4], F32, isOutput=False)
    b_kvu_k_d = nc.declare_dram_parameter("b_kvu_k", [128, 4], F32, isOutput=False)
    b_kvu_v_d = nc.declare_dram_parameter("b_kvu_v", [128, 512], F32, isOutput=False)
    b_o_d = nc.declare_dram_parameter("b_o", [128, DIM], F32, isOutput=False)
    out_d = nc.declare_dram_parameter("out", [S, DIM], F32, isOutput=True)

    with tile.TileContext(nc) as tc:
        with (
            tc.tile_pool(name="const", bufs=1) as const,
            tc.tile_pool(name="wts", bufs=1) as wts,
            tc.tile_pool(name="big", bufs=1) as big,
            tc.tile_pool(name="xin", bufs=4) as xin,
            tc.tile_pool(name="kvq", bufs=2) as kvq,
            tc.tile_pool(name="exd", bufs=6) as exd,
            tc.tile_pool(name="nrm", bufs=2) as nrm,
            tc.tile_pool(name="outs", bufs=3) as outs,
            tc.tile_pool(name="mps", bufs=2, space="PSUM") as mps,
        ):
            # ---- weight tiles (DMA'd inside W1, after the x DMAs) ----------
            w_kvc_sb = wts.tile([128, DIM], BF16, name="w_kvc_sb")
            w_qc_sb = wts.tile([128, ND * QR], BF16, name="w_qc_sb")
            w_kvu_k_sb = wts.tile([128, 512], BF16, name="w_kvu_k_sb")
            w_kvu_v_sb = wts.tile([128, 512], BF16, name="w_kvu_v_sb")
            w_qu_sb = wts.tile([128, 1024], BF16, name="w_qu_sb")
            w_o_sb = wts.tile([128, 4 * DIM], BF16, name="w_o_sb")
            b_kvc_sb = wts.tile([128, 1], F32, name="b_kvc_sb")
            b_qc_sb = wts.tile([128, 2], F32, name="b_qc_sb")
            b_qu_sb = wts.tile([128, 4], F32, name="b_qu_sb")
            b_kvu_k_sb = wts.tile([128, 4], F32, name="b_kvu_k_sb")
            b_kvu_v_sb = wts.tile([128, 512], F32, name="b_kvu_v_sb")
            b_o_sb = wts.tile([128, DIM], F32, name="b_o_sb")

            def emit_wdma_early():
                # merged DMAs (one instruction each); only what the latent
                # projections need right away.
                nc.sync.dma_start(
                    out=w_kvc_sb[:].rearrange("p (d c) -> p d c", c=128),
                    in_=w_kvc_d[:, :].rearrange("(d p) c -> p d c", p=128))
                nc.sync.dma_start(out=b_kvc_sb[:], in_=b_kvc_d[:, :])
                nc.sync.dma_start(
                    out=w_qc_sb[:].rearrange("p (d c) -> p d c", c=QR),
                    in_=w_qc_d[:, :].rearrange("(d p) c -> p d c", p=128))
                nc.sync.dma_start(out=b_qc_sb[:], in_=b_qc_d[:, :])

            def emit_wdma_mid():
                nc.sync.dma_start(out=w_kvu_k_sb[:], in_=w_kvu_k_d[:, :])
                nc.sync.dma_start(
                    out=w_qu_sb[:].rearrange("p (d c) -> p d c", c=512),
                    in_=w_qu_d[:, :].rearrange("(d p) c -> p d c", p=128))
                nc.sync.dma_start(out=b_qu_sb[:], in_=b_qu_d[:, :])
                nc.sync.dma_start(out=b_kvu_k_sb[:], in_=b_kvu_k_d[:, :])
                nc.sync.dma_start(out=w_kvu_v_sb[:], in_=w_kvu_v_d[:, :])
                nc.sync.dma_start(out=b_kvu_v_sb[:], in_=b_kvu_v_d[:, :])

            def emit_wdma_late():
                nc.sync.dma_start(
                    out=w_o_sb[:].rearrange("p (d c) -> p d c", c=DIM),
                    in_=w_o_d[:, :].rearrange("(d p) c -> p d c", p=128))
                nc.sync.dma_start(out=b_o_sb[:], in_=b_o_d[:, :])

            # ---- persistent products ---------------------------------------
            KT = big.tile([128, 4 * S], BF16, name="KT")
            QT = big.tile([128, 4 * S], BF16, name="QT")
            V = big.tile([128, NT * 520], BF16, name="V")
            ctxT = big.tile([128, 4 * S], BF16, name="ctxT")
            v_view = V[:].rearrange("p (k h c) -> p k h c", h=NHL, c=65)
            # ones columns of V (col 64 of each 65-wide head block)
            ones_f = const.tile([128, NT * NHL], F32, name="ones_f")
            nc.gpsimd.memset(ones_f[:], 1.0)
            nc.vector.tensor_copy(
                v_view[:, :, :, 64:65],
                ones_f[:].rearrange("p (k h o) -> p k h o", h=NHL, o=1))
            # warm the Exp activation table while the DMAs run so the first
            # real exp doesn't pay the table load
            warm = const.tile([1, 1], F32, name="warm")
            nc.scalar.activation(warm[:], ones_f[0:1, 0:1], AF.Exp)

            # out_proj j1 partial accumulators (cc 0-2, filled in late W3)
            pps = [None] * 16

            def emit_op1_partial(si, o2, pool):
                ps = pool.tile([128, 512], F32, tag="mm")
                for cc in range(3):
                    nc.tensor.matmul(
                        ps[:],
                        ctxT[:, cc * S + 128 * si:cc * S + 128 * si + 128],
                        w_o_sb[:, DIM * cc + o2:DIM * cc + o2 + 512],
                        start=cc == 0, stop=cc == 2)
                t = wts.tile([128, 512], F32, name=f"pp{si}_{o2}")
                pps[2 * (si - 8) + (o2 // 512)] = t
                nc.vector.tensor_add(t[:], ps[:], b_o_sb[:, o2:o2 + 512])

            def emit_op1_final(si, o2, pool):
                if o2 == 0:
                    obs[si % 4] = outs.tile([128, DIM], F32, tag="ob",
                                            name=f"ob{si}")
                ps = pool.tile([128, 512], F32, tag="mm")
                nc.tensor.matmul(
                    ps[:],
                    ctxT[:, 3 * S + 128 * si:3 * S + 128 * si + 128],
                    w_o_sb[:, DIM * 3 + o2:DIM * 3 + o2 + 512],
                    start=True, stop=True)
                ob = obs[si % 4]
                nc.vector.scalar_tensor_tensor(
                    ob[:, o2:o2 + 512], ps[:], 1.0,
                    pps[2 * (si - 8) + (o2 // 512)][:],
                    op0=mybir.AluOpType.mult, op1=mybir.AluOpType.add)
                nc.sync.dma_start(
                    out=out_d[128 * si:128 * si + 128, o2:o2 + 512],
                    in_=ob[:, o2:o2 + 512])

            # ---- projection emitters ---------------------------------------
            xts = [None] * NP
            kvs = [None] * NP
            q0s = [None] * NP
            q1s = [None] * NP

            def emit_xdma(p, dlo=0, dhi=ND):
                if dlo == 0:
                    xts[p] = xin.tile([128, ND * 512], BF16, tag="xt",
                                      name=f"xt{p}")
                xt = xts[p]
                nc.sync.dma_start(
                    out=xt[:, 512 * dlo:512 * dhi].rearrange(
                        "p (d s) -> p d s", s=512),
                    in_=xT_d[128 * dlo:128 * dhi, 512 * p:512 * p + 512]
                    .rearrange("(d p) s -> p d s", p=128))

            lat_open = {}

            def _lat_mm(kind, p, pool, rng, ps):
                for dc in rng:
                    if kind == "kv":
                        w = w_kvc_sb[:, 128 * dc:128 * dc + 128]
                    else:
                        half = int(kind[1])
                        w = w_qc_sb[:, QR * dc + 128 * half:
                                    QR * dc + 128 * half + 128]
                    nc.tensor.matmul(ps[:], w,
                                     xts[p][:, 512 * dc:512 * dc + 512],
                                     start=dc == 0, stop=dc == ND - 1)

            def _lat_drain(kind, p, ps):
                if kind == "kv":
                    t = kvq.tile([128, 512], BF16, tag="kvs", name=f"kvs{p}")
                    kvs[p] = t
                    nc.vector.tensor_scalar_add(t[:], ps[:], b_kvc_sb[:, 0:1])
                else:
                    half = int(kind[1])
                    t = kvq.tile([128, 512], BF16, tag=f"q{half}s",
                                 name=f"q{half}s{p}")
                    (q0s if half == 0 else q1s)[p] = t
                    nc.vector.tensor_scalar_add(t[:], ps[:],
                                                b_qc_sb[:, half:half + 1])

            def emit_lat(kind, p, pool):
                ps = pool.tile([128, 512], F32, tag="mm", name=f"lat{kind}{p}")
                _lat_mm(kind, p, pool, range(ND), ps)
                _lat_drain(kind, p, ps)

            def emit_lat_half(kind, p, second):
                # split fill: 4 accumulation matmuls per drain step, using a
                # dedicated psum slot so the open group survives interleaved
                # "mm"-tag fills
                # the two halves are always adjacent in the fill list, so at
                # most this one other "mm" alloc can slot in while the
                # accumulation group is open — safe with bufs=2
                if not second:
                    ps = mps.tile([128, 512], F32, tag="mm", bufs=2,
                                  name=f"lat{kind}{p}")
                    lat_open[(kind, p)] = ps
                    _lat_mm(kind, p, mps, range(0, 4), ps)
                else:
                    ps = lat_open.pop((kind, p))
                    _lat_mm(kind, p, mps, range(4, ND), ps)
                    _lat_drain(kind, p, ps)

            def emit_lat_kv(p, pool):
                emit_lat("kv", p, pool)

            def emit_lat_q(p, half, pool):
                emit_lat(f"q{half}", p, pool)

            def emit_qt(p, c, pool):
                ps = pool.tile([128, 512], F32, tag="mm")
                nc.tensor.matmul(ps[:], w_qu_sb[:, 128 * c:128 * c + 128],
                                 q0s[p][:], start=True, stop=False)
                nc.tensor.matmul(ps[:], w_qu_sb[:, 512 + 128 * c:512 + 128 * c + 128],
                                 q1s[p][:], start=False, stop=True)
                nc.vector.tensor_scalar_add(
                    QT[:, c * S + 512 * p:c * S + 512 * p + 512], ps[:],
                    b_qu_sb[:, c:c + 1])

            def emit_kt(p, c, pool):
                ps = pool.tile([128, 512], F32, tag="mm")
                nc.tensor.matmul(ps[:], w_kvu_k_sb[:, 128 * c:128 * c + 128],
                                 kvs[p][:], start=True, stop=True)
                nc.vector.tensor_scalar_add(
                    KT[:, c * S + 512 * p:c * S + 512 * p + 512], ps[:],
                    b_kvu_k_sb[:, c:c + 1])

            def emit_v(p, q, pool):
                k = 4 * p + q
                ps = pool.tile([128, 512], F32, tag="mm")
                nc.tensor.matmul(ps[:], kvs[p][:, 128 * q:128 * q + 128],
                                 w_kvu_v_sb[:], start=True, stop=True)
                nc.vector.tensor_add(
                    v_view[:, k, :, 0:64],
                    ps[:].rearrange("p (h c) -> p h c", c=64),
                    b_kvu_v_sb[:].rearrange("p (h c) -> p h c", c=64))

            obs = [None] * 4

            def emit_outproj_half(si, o2, pool):
                if o2 == 0:
                    obs[si % 4] = outs.tile([128, DIM], F32, tag="ob",
                                            name=f"ob{si}")
                ps = pool.tile([128, 512], F32, tag="mm")
                for cc in range(4):
                    nc.tensor.matmul(
                        ps[:],
                        ctxT[:, cc * S + 128 * si:cc * S + 128 * si + 128],
                        w_o_sb[:, DIM * cc + o2:DIM * cc + o2 + 512],
                        start=cc == 0, stop=cc == 3)
                ob = obs[si % 4]
                nc.vector.tensor_add(ob[:, o2:o2 + 512], ps[:],
                                     b_o_sb[:, o2:o2 + 512])
                nc.sync.dma_start(
                    out=out_d[128 * si:128 * si + 128, o2:o2 + 512],
                    in_=ob[:, o2:o2 + 512])

            # ---- attention ------------------------------------------------
            # One flat pipelined stream over all (j, h) passes.  PV is
            # emitted DEPTH iterations late and the pipeline crosses head
            # boundaries, so the next head's QK is always ahead of the
            # previous head's trailing PVs — the scalar engine (exp) is the
            # attention bottleneck and must never wait on PV/normalize
            # chains.  Each head's normalize is emitted when its last PV
            # pops.  fill() interleaves projection/out_proj PE work; need
            # hooks drain fills an iteration depends on.
            DEPTH = 3

            def attn_stream(scp, ctp, segments):
                pend = []

                def pop():
                    pv, norm = pend.pop(0)
                    pv()
                    if norm is not None:
                        norm()

                for j, h, fill, need in segments:
                    s0 = SH * j
                    c = h // 2
                    po = 64 * (h % 2)
                    kmax = (s0 + SH) // 128
                    last_k = {bi: min(kmax - 1,
                                      (s0 + 512 * (bi + 1)) // 128 - 1)
                              for bi in range(SH // 512)}
                    ctx = ctp.tile([65, SH], F32, tag="ctx", name=f"ctx{j}{h}")

                    def emit_pv(k, relc, ex, exo, ctx=ctx, h=h, last_k=last_k):
                        for bi in range(SH // 512):
                            a = max(relc, 512 * bi)
                            b2 = min(SH, 512 * bi + 512)
                            if a >= b2:
                                continue
                            nc.tensor.matmul(
                                ctx[:, a:b2],
                                V[:, 520 * k + 65 * h:520 * k + 65 * h + 65],
                                ex[:, exo + a - relc:exo + b2 - relc],
                                start=(k == 0), stop=(k == last_k[bi]))

                    def emit_norm(bi, ctx=ctx, c=c, po=po, s0=s0):
                        # normalize bank bi: ctx[0:64] * (1/ctx[64]) -> ctxT.
                        # Emitted as soon as that bank's accumulation closes
                        # (last_k[bi]) so most of the chain overlaps the
                        # remaining iterations of the same head.
                        o2 = 512 * bi
                        rec = nrm.tile([1, 512], F32, tag="rec")
                        nc.vector.reciprocal(rec[:], ctx[64:65, o2:o2 + 512])
                        rbc = nrm.tile([64, 512], F32, tag="rbc")
                        nc.gpsimd.partition_broadcast(rbc[:], rec[0:1, :])
                        nc.vector.tensor_mul(
                            ctxT[po:po + 64,
                                 c * S + s0 + o2:c * S + s0 + o2 + 512],
                            ctx[0:64, o2:o2 + 512], rbc[:])

                    # group trailing small-window k's so several share one
                    # sc tile and ONE exp instruction (fewer activation
                    # access-latency bubbles and sem chains)
                    groups = []
                    cur, acc = [], 0
                    for k in range(kmax):
                        fd_k = s0 + SH - max(s0, 128 * k)
                        if cur and acc + fd_k <= SH:
                            cur.append(k)
                            acc += fd_k
                        else:
                            if cur:
                                groups.append(cur)
                            cur, acc = [k], fd_k
                    groups.append(cur)

                    for grp in groups:
                        if need is not None:
                            for k in grp:
                                need(k)
                        sc = scp.tile([128, SH], F32, tag="sc")
                        meta = []
                        off = 0
                        for k in grp:
                            t0 = 128 * k
                            ss = max(s0, t0)
                            fd = s0 + SH - ss
                            relc = ss - s0
                            meta.append((k, t0, relc, fd, off))
                            for o2, w2 in _pieces(fd):
                                nc.tensor.matmul(
                                    sc[:, off + o2:off + o2 + w2],
                                    KT[po:po + 64,
                                       c * S + t0:c * S + t0 + 128],
                                    QT[po:po + 64,
                                       c * S + ss + o2:c * S + ss + o2 + w2],
                                    start=True, stop=True)
                            off += fd
                        ex = exd.tile([128, SH], BF16, tag="ex")
                        nc.scalar.activation(ex[:, :off], sc[:, :off], AF.Exp,
                                             scale=0.125)
                        for k, t0, relc, fd, exo in meta:
                            if t0 >= s0:
                                nc.gpsimd.affine_select(
                                    out=ex[:, exo:exo + 128],
                                    in_=ex[:, exo:exo + 128],
                                    pattern=[[1, 128]],
                                    compare_op=mybir.AluOpType.is_ge,
                                    fill=0.0, base=0, channel_multiplier=-1)
                        for k, t0, relc, fd, exo in meta:
                            if len(pend) == DEPTH:
                                pop()
                            norms = [bi for bi in range(SH // 512)
                                     if last_k[bi] == k]
                            pend.append((
                                lambda k=k, relc=relc, ex=ex, exo=exo,
                                f=emit_pv: f(k, relc, ex, exo),
                                (lambda bis=norms, f=emit_norm:
                                 [f(bi) for bi in bis]) if norms else None))
                        fill()
                while pend:
                    pop()

            # ================= W1: minimal projection prefix =================
            # Just enough that head 0's j=0 attention can start: q-latents for
            # pieces 0,1, the c=0 K/Q up-projections, V chunk 0.  Everything
            # else becomes interleaved fill work.
            with tc.tile_pool(name="w1p", bufs=4, space="PSUM") as w1p:
                emit_xdma(0, 0, 4)
                emit_wdma_early()
                emit_xdma(0, 4, ND)
                emit_xdma(1)
                emit_wdma_mid()
                emit_xdma(2)
                emit_xdma(3)
                emit_wdma_late()
                emit_lat_kv(0, w1p)
                emit_lat_q(0, 0, w1p)
                emit_lat_q(0, 1, w1p)
                emit_lat_q(1, 0, w1p)
                emit_lat_q(1, 1, w1p)
                emit_kt(0, 0, w1p)
                emit_qt(0, 0, w1p)
                emit_qt(1, 0, w1p)
                emit_v(0, 0, w1p)

            # Unified interleaved fill stream, in dependency order; labels
            # mark the last step each attention point requires.  The j=0
            # window drains only up to CAP (its own needs); the p2/p3
            # projections and j=0 out_proj drain lazily through the j=1
            # window, where the scalar engine is the bottleneck and the PE
            # has idle slack.
            fills = []

            def F(label, fn):
                fills.append((label, fn))

            for sec in (False, True):
                F("h0i1", lambda sec=sec: emit_lat_half("kv", 1, sec))
            F("h0i2", lambda: emit_v(0, 1, mps))
            F("h0i3", lambda: emit_v(0, 2, mps))
            F("h0i4", lambda: emit_kt(1, 0, mps))
            F("h0i4", lambda: emit_v(0, 3, mps))
            F("h0i5", lambda: emit_v(1, 0, mps))
            F("h0i6", lambda: emit_v(1, 1, mps))
            F("h0i7", lambda: emit_v(1, 2, mps))
            F("h0i7", lambda: emit_v(1, 3, mps))
            for c in (1, 2, 3):
                for p in (0, 1):
                    F(f"kq{c}", lambda p=p, c=c: emit_qt(p, c, mps))
                    F(f"kq{c}", lambda p=p, c=c: emit_kt(p, c, mps))
            for p in (2, 3):
                for kind in ("kv", "q0", "q1"):
                    for sec in (False, True):
                        F("j1lat", lambda p=p, kind=kind, sec=sec:
                          emit_lat_half(kind, p, sec))
            F("j1q0", lambda: emit_qt(2, 0, mps))
            F("j1q0", lambda: emit_qt(3, 0, mps))
            F("j1k8c0", lambda: emit_kt(2, 0, mps))
            for q in range(4):
                F("j1v2", lambda q=q: emit_v(2, q, mps))
            F("j1k12c0", lambda: emit_kt(3, 0, mps))
            for q in range(4):
                F("j1v3", lambda q=q: emit_v(3, q, mps))
            for c in (1, 2, 3):
                F(f"j1q{c}", lambda c=c: emit_qt(2, c, mps))
                F(f"j1q{c}", lambda c=c: emit_qt(3, c, mps))
                F(f"j1k8c{c}", lambda c=c: emit_kt(2, c, mps))
                F(f"j1k12c{c}", lambda c=c: emit_kt(3, c, mps))
            for si in range(8):
                F("op0", lambda si=si: emit_outproj_half(si, 0, mps))
                F("op0", lambda si=si: emit_outproj_half(si, 512, mps))

            fill_pos = [0]
            CAP = max(i for i, (lb, _) in enumerate(fills) if lb == "j1q0") + 1

            def drain(n, cap=None):
                lim = len(fills) if cap is None else cap
                while n > 0 and fill_pos[0] < lim:
                    fills[fill_pos[0]][1]()
                    fill_pos[0] += 1
                    n -= 1

            def drain_until(label):
                idx = max((i for i, (lb, _) in enumerate(fills) if lb == label),
                          default=-1)
                while fill_pos[0] <= idx:
                    fills[fill_pos[0]][1]()
                    fill_pos[0] += 1

            def need_j0(h):
                def need(k):
                    if h == 0 and 1 <= k <= 7:
                        drain_until(f"h0i{k}")
                    if k == 0:
                        if h == 1:
                            drain_until("h0i7")
                        elif h >= 2:
                            drain_until(f"kq{h // 2}")
                return need

            def need_j1(h):
                c = h // 2

                def need(k):
                    if k == 0:
                        drain_until(f"j1q{c}")
                    elif k == 8:
                        drain_until("j1v2" if c == 0 else f"j1k8c{c}")
                    elif k == 12:
                        drain_until("j1v3" if c == 0 else f"j1k12c{c}")
                return need

            segs = []
            for h in range(NHL):
                segs.append((0, h, lambda: drain(1, cap=CAP), need_j0(h)))
            for h in range(NHL):
                segs.append((1, h, lambda: drain(1), need_j1(h)))

            with (
                tc.tile_pool(name="scp", bufs=2, space="PSUM") as scp,
                tc.tile_pool(name="ctp", bufs=1, space="PSUM") as ctp,
            ):
                attn_stream(scp, ctp, segs)
                drain(len(fills))

            # ================= W4: out_proj j1 tail =========================
            for si in range(8, 16):
                emit_outproj_half(si, 0, mps)
                emit_outproj_half(si, 512, mps)

    nc.finalize()
    return nc


def shard_inputs(inputs, S=2048):
    """Build the 8 per-core input maps from full inputs (host-side prep:
    transpose x, cast matmul operands to bf16, pre-broadcast row biases)."""
    import ml_dtypes
    bf = lambda a: np.ascontiguousarray(np.asarray(a)).astype(ml_dtypes.bfloat16)
    f = lambda a: np.ascontiguousarray(np.asarray(a, dtype=np.float32))
    x = np.asarray(inputs["x"], dtype=np.float32)
    w_kvc, b_kvc = inputs["w_kvc"], f(inputs["b_kvc"])
    w_kvu, b_kvu = np.asarray(inputs["w_kvu"]), f(inputs["b_kvu"])
    w_qc, b_qc = inputs["w_qc"], f(inputs["b_qc"])
    w_qu, b_qu = np.asarray(inputs["w_qu"]), f(inputs["b_qu"])
    w_o, b_o = np.asarray(inputs["w_o"]), f(inputs["b_o"])
    xT = [bf(x[b].T) for b in range(B)]
    w_kvc_b = bf(w_kvc)
    w_qc_b = bf(w_qc)
    in_maps = []
    for core in range(NCORES):
        b = core // 2
        g = core % 2
        cs = slice(512 * g, 512 * g + 512)
        in_maps.append({
            "xT": xT[b],
            "w_kvc": w_kvc_b,
            "w_qc": w_qc_b,
            "w_kvu_k": bf(w_kvu[:, cs]),
            "w_kvu_v": bf(w_kvu[:, 1024 + 512 * g:1024 + 512 * g + 512]),
            "w_qu": bf(w_qu[:, cs]),
            "w_o": bf(w_o[cs, :]),
            "b_kvc": b_kvc.reshape(LAT, 1),
            "b_qc": np.ascontiguousarray(b_qc.reshape(2, 128).T),
            "b_qu": np.ascontiguousarray(b_qu[cs].reshape(4, 128).T),
            "b_kvu_k": np.ascontiguousarray(b_kvu[cs].reshape(4, 128).T),
            "b_kvu_v": np.ascontiguousarray(np.broadcast_to(
                b_kvu[1024 + 512 * g:1024 + 512 * g + 512], (128, 512))),
            "b_o": np.ascontiguousarray(np.broadcast_to(b_o * 0.5, (128, DIM))),
        })
    return in_maps


def kernel(**inputs) -> np.ndarray:
    from concourse.bass_utils import run_bass_kernel_spmd

    x = np.asarray(inputs["x"])
    S = x.shape[1]
    nc = build_mla(S=S)
    in_maps = shard_inputs(inputs, S=S)
    res = run_bass_kernel_spmd(nc, in_maps, list(range(NCORES))).results
    out = np.empty((B, S, DIM), dtype=np.float32)
    for b in range(B):
        out[b] = res[2 * b]["out"] + res[2 * b + 1]["out"]
    return out


# revision 62
# speedup vs baseline: 1.0008x; 1.0008x over previous
"""MLA (multi-head latent attention) Bass kernel for Trainium2, 8 NeuronCores.

Sharding: core i handles batch b = i // 2 and head-group g = i % 2
(8 of the 16 heads).  Each core computes a partial output
(its heads' contribution through out_proj, plus b_o/2); the host sums
the two partials per batch.

v2 design notes (all driven by the CoreSim v1 cost model):
  - The host pre-transposes x to [dim, S] and pre-casts x + all weights
    to bf16, so there are no on-chip PE transposes and no weight staging
    copies.  bf16 matmuls are 1 cycle/row at any moving width (f32r
    degrades 4x below 256), and halve SBUF/DMA traffic.
  - Row-vector biases (b_kvu_v, b_o) arrive pre-broadcast as [128, n]
    tiles and are folded into the PSUM->SBUF drain via tensor_add on the
    DVE, so the PE never runs bias matmuls.  Per-partition biases
    (b_kvc/b_qc/b_qu/b_kvu_k) fold into tensor_scalar_add drains.
  - Layout: everything "t-major" (feature dim on partitions):
      xT [8x128, S] (DMA'd), kv_latT [128,S], q_latT [256,S],
      KT/QT [128 = 2 heads x 64, 4 chunks x S] bf16,
      V [128 tokens, 16 chunks x (8 heads x 65)] bf16 with a ones
      column per head (col 64) that accumulates the softmax denominator
      during the PV matmul (row 64 of ctx PSUM).
  - Attention per (s-half j, head h): stream key chunks k; QK scores to
    PSUM, exp(s/8) on the scalar engine (the only Activation-engine
    work in the kernel), causal diagonal via gpsimd affine_select, PV
    accumulates ctx^T[65, SH] in PSUM; normalize via DVE reciprocal +
    gpsimd partition_broadcast + DVE multiply into ctxT bf16.
  - Software pipelining by emission order: a minimal projection prefix,
    then attention j=0 interleaved with the remaining projection work,
    then attention j=1 interleaved with out_proj of the first half,
    then the out_proj tail.  Interleaved "fill" work keeps the PE busy
    while the scalar engine (the attention-phase bottleneck) runs exp.
  - PSUM budget (8 banks): sc slots 2x[128,1024]f32 = 4, ctx
    [65,1024]f32 = 2, shared misc pool [128,512]f32 x2 = 2.
"""

import numpy as np

import concourse.bass as bass
import concourse.bacc as bacc
import concourse.mybir as mybir
import concourse.tile as tile

DIM = 1024
NUM_HEADS = 16
HEAD_DIM = 64
LAT = 128
QR = 256
B = 4
NCORES = 8
ND = DIM // 128       # 8 d-chunks
NHL = 8               # heads per core
F32 = mybir.dt.float32
BF16 = mybir.dt.bfloat16
AF = mybir.ActivationFunctionType


def _pieces(total, w=512):
    return [(o, min(w, total - o)) for o in range(0, total, w)]


def build_mla(S=2048):
    """Build the per-core Bass program (same SPMD program on all 8 cores)."""
    assert S % 1024 == 0
    SH = S // 2           # s-half width
    NT = S // 128         # number of 128-token chunks
    NP = S // 512         # number of 512-token projection pieces

    nc = bacc.Bacc()

    xT_d = nc.declare_dram_parameter("xT", [DIM, S], BF16, isOutput=False)
    w_kvc_d = nc.declare_dram_parameter("w_kvc", [DIM, LAT], BF16, isOutput=False)
    w_qc_d = nc.declare_dram_parameter("w_qc", [DIM, QR], BF16, isOutput=False)
    w_kvu_k_d = nc.declare_dram_parameter("w_kvu_k", [LAT, 512], BF16, isOutput=False)
    w_kvu_v_d = nc.declare_dram_parameter("w_kvu_v", [LAT, 512], BF16, isOutput=False)
    w_qu_d = nc.declare_dram_parameter("w_qu", [QR, 512], BF16, isOutput=False)
    w_o_d = nc.declare_dram_parameter("w_o", [512, DIM], BF16, isOutput=False)
    b_kvc_d = nc.declare_dram_parameter("b_kvc", [LAT, 1], F32, isOutput=False)
    b_qc_d = nc.declare_dram_parameter("b_qc", [128, 2], F32, isOutput=False)
    b_qu_d = nc.declare_dram_parameter("b_qu", [128, 4], F32, isOutput=False)
    b_kvu_k_d = nc.declare_dram_parameter("b_kvu_k", [128, 4], F32, isOutput=False)
    b_kvu_v_d = nc.declare_dram_parameter("b_kvu_v", [128, 512], F32, isOutput=False)
    b_o_d = nc.declare_dram_parameter("b_o", [128, DIM], F32, isOutput=False)
    out_d = nc.declare_dram_parameter("out", [S, DIM], F32, isOutput=True)

    with tile.TileContext(nc) as tc:
        with (
            tc.tile_pool(name="const", bufs=1) as const,
            tc.tile_pool(name="wts", bufs=1) as wts,
            tc.tile_pool(name="big", bufs=1) as big,
            tc.tile_pool(name="xin", bufs=4) as xin,
            tc.tile_pool(name="kvq", bufs=2) as kvq,
            tc.tile_pool(name="exd", bufs=8) as exd,
            tc.tile_pool(name="nrm", bufs=2) as nrm,
            tc.tile_pool(name="outs", bufs=3) as outs,
            tc.tile_pool(name="mps", bufs=2, space="PSUM") as mps,
        ):
            # ---- weight tiles (DMA'd inside W1, after the x DMAs) ----------
            w_kvc_sb = wts.tile([128, DIM], BF16, name="w_kvc_sb")
            w_qc_sb = wts.tile([128, ND * QR], BF16, name="w_qc_sb")
            w_kvu_k_sb = wts.tile([128, 512], BF16, name="w_kvu_k_sb")
            w_kvu_v_sb = wts.tile([128, 512], BF16, name="w_kvu_v_sb")
            w_qu_sb = wts.tile([128, 1024], BF16, name="w_qu_sb")
            w_o_sb = wts.tile([128, 4 * DIM], BF16, name="w_o_sb")
            b_kvc_sb = wts.tile([128, 1], F32, name="b_kvc_sb")
            b_qc_sb = wts.tile([128, 2], F32, name="b_qc_sb")
            b_qu_sb = wts.tile([128, 4], F32, name="b_qu_sb")
            b_kvu_k_sb = wts.tile([128, 4], F32, name="b_kvu_k_sb")
            b_kvu_v_sb = wts.tile([128, 512], F32, name="b_kvu_v_sb")
            b_o_sb = wts.tile([128, DIM], F32, name="b_o_sb")

            def emit_wdma_early():
                # merged DMAs (one instruction each); only what the latent
                # projections need right away.
                nc.sync.dma_start(
                    out=w_kvc_sb[:].rearrange("p (d c) -> p d c", c=128),
                    in_=w_kvc_d[:, :].rearrange("(d p) c -> p d c", p=128))
                nc.sync.dma_start(out=b_kvc_sb[:], in_=b_kvc_d[:, :])
                nc.sync.dma_start(
                    out=w_qc_sb[:].rearrange("p (d c) -> p d c", c=QR),
                    in_=w_qc_d[:, :].rearrange("(d p) c -> p d c", p=128))
                nc.sync.dma_start(out=b_qc_sb[:], in_=b_qc_d[:, :])

            def emit_wdma_mid():
                nc.sync.dma_start(out=w_kvu_k_sb[:], in_=w_kvu_k_d[:, :])
                nc.sync.dma_start(
                    out=w_qu_sb[:].rearrange("p (d c) -> p d c", c=512),
                    in_=w_qu_d[:, :].rearrange("(d p) c -> p d c", p=128))
                nc.sync.dma_start(out=b_qu_sb[:], in_=b_qu_d[:, :])
                nc.sync.dma_start(out=b_kvu_k_sb[:], in_=b_kvu_k_d[:, :])
                nc.sync.dma_start(out=w_kvu_v_sb[:], in_=w_kvu_v_d[:, :])
                nc.sync.dma_start(out=b_kvu_v_sb[:], in_=b_kvu_v_d[:, :])

            def emit_wdma_late():
                nc.sync.dma_start(
                    out=w_o_sb[:].rearrange("p (d c) -> p d c", c=DIM),
                    in_=w_o_d[:, :].rearrange("(d p) c -> p d c", p=128))
                nc.sync.dma_start(out=b_o_sb[:], in_=b_o_d[:, :])

            # ---- persistent products ---------------------------------------
            KT = big.tile([128, 4 * S], BF16, name="KT")
            QT = big.tile([128, 4 * S], BF16, name="QT")
            V = big.tile([128, NT * 520], BF16, name="V")
            ctxT = big.tile([128, 4 * S], BF16, name="ctxT")
            v_view = V[:].rearrange("p (k h c) -> p k h c", h=NHL, c=65)
            # ones columns of V (col 64 of each 65-wide head block)
            ones_f = const.tile([128, NT * NHL], F32, name="ones_f")
            nc.gpsimd.memset(ones_f[:], 1.0)
            nc.vector.tensor_copy(
                v_view[:, :, :, 64:65],
                ones_f[:].rearrange("p (k h o) -> p k h o", h=NHL, o=1))
            # warm the Exp activation table while the DMAs run so the first
            # real exp doesn't pay the table load
            warm = const.tile([1, 1], F32, name="warm")
            nc.scalar.activation(warm[:], ones_f[0:1, 0:1], AF.Exp)

            # out_proj j1 partial accumulators (cc 0-2, filled in late W3)
            pps = [None] * 16

            def emit_op1_partial(si, o2, pool):
                ps = pool.tile([128, 512], F32, tag="mm")
                for cc in range(3):
                    nc.tensor.matmul(
                        ps[:],
                        ctxT[:, cc * S + 128 * si:cc * S + 128 * si + 128],
                        w_o_sb[:, DIM * cc + o2:DIM * cc + o2 + 512],
                        start=cc == 0, stop=cc == 2)
                t = wts.tile([128, 512], F32, name=f"pp{si}_{o2}")
                pps[2 * (si - 8) + (o2 // 512)] = t
                nc.vector.tensor_add(t[:], ps[:], b_o_sb[:, o2:o2 + 512])

            def emit_op1_final(si, o2, pool):
                if o2 == 0:
                    obs[si % 4] = outs.tile([128, DIM], F32, tag="ob",
                                            name=f"ob{si}")
                ps = pool.tile([128, 512], F32, tag="mm")
                nc.tensor.matmul(
                    ps[:],
                    ctxT[:, 3 * S + 128 * si:3 * S + 128 * si + 128],
                    w_o_sb[:, DIM * 3 + o2:DIM * 3 + o2 + 512],
                    start=True, stop=True)
                ob = obs[si % 4]
                nc.vector.scalar_tensor_tensor(
                    ob[:, o2:o2 + 512], ps[:], 1.0,
                    pps[2 * (si - 8) + (o2 // 512)][:],
                    op0=mybir.AluOpType.mult, op1=mybir.AluOpType.add)
                nc.sync.dma_start(
                    out=out_d[128 * si:128 * si + 128, o2:o2 + 512],
                    in_=ob[:, o2:o2 + 512])

            # ---- projection emitters ---------------------------------------
            xts = [None] * NP
            kvs = [None] * NP
            q0s = [None] * NP
            q1s = [None] * NP

            def emit_xdma(p, dlo=0, dhi=ND):
                if dlo == 0:
                    xts[p] = xin.tile([128, ND * 512], BF16, tag="xt",
                                      name=f"xt{p}")
                xt = xts[p]
                nc.sync.dma_start(
                    out=xt[:, 512 * dlo:512 * dhi].rearrange(
                        "p (d s) -> p d s", s=512),
                    in_=xT_d[128 * dlo:128 * dhi, 512 * p:512 * p + 512]
                    .rearrange("(d p) s -> p d s", p=128))

            lat_open = {}

            def _lat_mm(kind, p, pool, rng, ps):
                for dc in rng:
                    if kind == "kv":
                        w = w_kvc_sb[:, 128 * dc:128 * dc + 128]
                    else:
                        half = int(kind[1])
                        w = w_qc_sb[:, QR * dc + 128 * half:
                                    QR * dc + 128 * half + 128]
                    nc.tensor.matmul(ps[:], w,
                                     xts[p][:, 512 * dc:512 * dc + 512],
                                     start=dc == 0, stop=dc == ND - 1)

            def _lat_drain(kind, p, ps):
                if kind == "kv":
                    t = kvq.tile([128, 512], BF16, tag="kvs", name=f"kvs{p}")
                    kvs[p] = t
                    nc.vector.tensor_scalar_add(t[:], ps[:], b_kvc_sb[:, 0:1])
                else:
                    half = int(kind[1])
                    t = kvq.tile([128, 512], BF16, tag=f"q{half}s",
                                 name=f"q{half}s{p}")
                    (q0s if half == 0 else q1s)[p] = t
                    nc.vector.tensor_scalar_add(t[:], ps[:],
                                                b_qc_sb[:, half:half + 1])

            def emit_lat(kind, p, pool):
                ps = pool.tile([128, 512], F32, tag="mm", name=f"lat{kind}{p}")
                _lat_mm(kind, p, pool, range(ND), ps)
                _lat_drain(kind, p, ps)

            def emit_lat_half(kind, p, second):
                # split fill: 4 accumulation matmuls per drain step, using a
                # dedicated psum slot so the open group survives interleaved
                # "mm"-tag fills
                # the two halves are always adjacent in the fill list, so at
                # most this one other "mm" alloc can slot in while the
                # accumulation group is open — safe with bufs=2
                if not second:
                    ps = mps.tile([128, 512], F32, tag="mm", bufs=2,
                                  name=f"lat{kind}{p}")
                    lat_open[(kind, p)] = ps
                    _lat_mm(kind, p, mps, range(0, 4), ps)
                else:
                    ps = lat_open.pop((kind, p))
                    _lat_mm(kind, p, mps, range(4, ND), ps)
                    _lat_drain(kind, p, ps)

            def emit_lat_kv(p, pool):
                emit_lat("kv", p, pool)

            def emit_lat_q(p, half, pool):
                emit_lat(f"q{half}", p, pool)

            def emit_qt(p, c, pool):
                ps = pool.tile([128, 512], F32, tag="mm")
                nc.tensor.matmul(ps[:], w_qu_sb[:, 128 * c:128 * c + 128],
                                 q0s[p][:], start=True, stop=False)
                nc.tensor.matmul(ps[:], w_qu_sb[:, 512 + 128 * c:512 + 128 * c + 128],
                                 q1s[p][:], start=False, stop=True)
                nc.vector.tensor_scalar_add(
                    QT[:, c * S + 512 * p:c * S + 512 * p + 512], ps[:],
                    b_qu_sb[:, c:c + 1])

            def emit_kt(p, c, pool):
                ps = pool.tile([128, 512], F32, tag="mm")
                nc.tensor.matmul(ps[:], w_kvu_k_sb[:, 128 * c:128 * c + 128],
                                 kvs[p][:], start=True, stop=True)
                nc.vector.tensor_scalar_add(
                    KT[:, c * S + 512 * p:c * S + 512 * p + 512], ps[:],
                    b_kvu_k_sb[:, c:c + 1])

            def emit_v(p, q, pool):
                k = 4 * p + q
                ps = pool.tile([128, 512], F32, tag="mm")
                nc.tensor.matmul(ps[:], kvs[p][:, 128 * q:128 * q + 128],
                                 w_kvu_v_sb[:], start=True, stop=True)
                nc.vector.tensor_add(
                    v_view[:, k, :, 0:64],
                    ps[:].rearrange("p (h c) -> p h c", c=64),
                    b_kvu_v_sb[:].rearrange("p (h c) -> p h c", c=64))

            obs = [None] * 4

            def emit_outproj_half(si, o2, pool):
                if o2 == 0:
                    obs[si % 4] = outs.tile([128, DIM], F32, tag="ob",
                                            name=f"ob{si}")
                ps = pool.tile([128, 512], F32, tag="mm")
                for cc in range(4):
                    nc.tensor.matmul(
                        ps[:],
                        ctxT[:, cc * S + 128 * si:cc * S + 128 * si + 128],
                        w_o_sb[:, DIM * cc + o2:DIM * cc + o2 + 512],
                        start=cc == 0, stop=cc == 3)
                ob = obs[si % 4]
                nc.vector.tensor_add(ob[:, o2:o2 + 512], ps[:],
                                     b_o_sb[:, o2:o2 + 512])
                nc.sync.dma_start(
                    out=out_d[128 * si:128 * si + 128, o2:o2 + 512],
                    in_=ob[:, o2:o2 + 512])

            # ---- attention ------------------------------------------------
            # One flat pipelined stream over all (j, h) passes.  PV is
            # emitted DEPTH iterations late and the pipeline crosses head
            # boundaries, so the next head's QK is always ahead of the
            # previous head's trailing PVs — the scalar engine (exp) is the
            # attention bottleneck and must never wait on PV/normalize
            # chains.  Each head's normalize is emitted when its last PV
            # pops.  fill() interleaves projection/out_proj PE work; need
            # hooks drain fills an iteration depends on.
            DEPTH = 3

            def attn_stream(scp, ctp, segments):
                pend = []

                def pop():
                    pv, norm = pend.pop(0)
                    pv()
                    if norm is not None:
                        norm()

                for j, h, fill, need in segments:
                    s0 = SH * j
                    c = h // 2
                    po = 64 * (h % 2)
                    kmax = (s0 + SH) // 128
                    last_k = {bi: min(kmax - 1,
                                      (s0 + 512 * (bi + 1)) // 128 - 1)
                              for bi in range(SH // 512)}
                    ctx = ctp.tile([65, SH], F32, tag="ctx", name=f"ctx{j}{h}")

                    def emit_pv(k, relc, ex, exo, ctx=ctx, h=h, last_k=last_k):
                        for bi in range(SH // 512):
                            a = max(relc, 512 * bi)
                            b2 = min(SH, 512 * bi + 512)
                            if a >= b2:
                                continue
                            nc.tensor.matmul(
                                ctx[:, a:b2],
                                V[:, 520 * k + 65 * h:520 * k + 65 * h + 65],
                                ex[:, exo + a - relc:exo + b2 - relc],
                                start=(k == 0), stop=(k == last_k[bi]))

                    def emit_norm(bi, ctx=ctx, c=c, po=po, s0=s0):
                        # normalize bank bi: ctx[0:64] * (1/ctx[64]) -> ctxT.
                        # Emitted as soon as that bank's accumulation closes
                        # (last_k[bi]) so most of the chain overlaps the
                        # remaining iterations of the same head.
                        o2 = 512 * bi
                        rec = nrm.tile([1, 512], F32, tag="rec")
                        nc.vector.reciprocal(rec[:], ctx[64:65, o2:o2 + 512])
                        rbc = nrm.tile([64, 512], F32, tag="rbc")
                        nc.gpsimd.partition_broadcast(rbc[:], rec[0:1, :])
                        nc.vector.tensor_mul(
                            ctxT[po:po + 64,
                                 c * S + s0 + o2:c * S + s0 + o2 + 512],
                            ctx[0:64, o2:o2 + 512], rbc[:])

                    # group trailing small-window k's so several share one
                    # sc tile and ONE exp instruction (fewer activation
                    # access-latency bubbles and sem chains)
                    groups = []
                    cur, acc = [], 0
                    for k in range(kmax):
                        fd_k = s0 + SH - max(s0, 128 * k)
                        if cur and acc + fd_k <= SH:
                            cur.append(k)
                            acc += fd_k
                        else:
                            if cur:
                                groups.append(cur)
                            cur, acc = [k], fd_k
                    groups.append(cur)

                    for grp in groups:
                        if need is not None:
                            for k in grp:
                                need(k)
                        sc = scp.tile([128, SH], F32, tag="sc")
                        meta = []
                        off = 0
                        for k in grp:
                            t0 = 128 * k
                            ss = max(s0, t0)
                            fd = s0 + SH - ss
                            relc = ss - s0
                            meta.append((k, t0, relc, fd, off))
                            for o2, w2 in _pieces(fd):
                                nc.tensor.matmul(
                                    sc[:, off + o2:off + o2 + w2],
                                    KT[po:po + 64,
                                       c * S + t0:c * S + t0 + 128],
                                    QT[po:po + 64,
                                       c * S + ss + o2:c * S + ss + o2 + w2],
                                    start=True, stop=True)
                            off += fd
                        ex = exd.tile([128, SH], BF16, tag="ex")
                        nc.scalar.activation(ex[:, :off], sc[:, :off], AF.Exp,
                                             scale=0.125)
                        for k, t0, relc, fd, exo in meta:
                            if t0 >= s0:
                                nc.gpsimd.affine_select(
                                    out=ex[:, exo:exo + 128],
                                    in_=ex[:, exo:exo + 128],
                                    pattern=[[1, 128]],
                                    compare_op=mybir.AluOpType.is_ge,
                                    fill=0.0, base=0, channel_multiplier=-1)
                        for k, t0, relc, fd, exo in meta:
                            if len(pend) == DEPTH:
                                pop()
                            norms = [bi for bi in range(SH // 512)
                                     if last_k[bi] == k]
                            pend.append((
                                lambda k=k, relc=relc, ex=ex, exo=exo,
                                f=emit_pv: f(k, relc, ex, exo),
                                (lambda bis=norms, f=emit_norm:
                                 [f(bi) for bi in bis]) if norms else None))
                        fill()
                while pend:
                    pop()

            # ================= W1: minimal projection prefix =================
            # Just enough that head 0's j=0 attention can start: q-latents for
            # pieces 0,1, the c=0 K/Q up-projections, V chunk 0.  Everything
            # else becomes interleaved fill work.
            with tc.tile_pool(name="w1p", bufs=4, space="PSUM") as w1p:
                emit_xdma(0, 0, 4)
                emit_wdma_early()
                emit_xdma(0, 4, ND)
                emit_xdma(1)
                emit_wdma_mid()
                emit_xdma(2)
                emit_xdma(3)
                emit_wdma_late()
                emit_lat_kv(0, w1p)
                emit_lat_q(0, 0, w1p)
                emit_lat_q(0, 1, w1p)
                emit_lat_q(1, 0, w1p)
                emit_lat_q(1, 1, w1p)
                emit_kt(0, 0, w1p)
                emit_qt(0, 0, w1p)
                emit_qt(1, 0, w1p)
                emit_v(0, 0, w1p)

            # Unified interleaved fill stream, in dependency order; labels
            # mark the last step each attention point requires.  The j=0
            # window drains only up to CAP (its own needs); the p2/p3
            # projections and j=0 out_proj drain lazily through the j=1
            # window, where the scalar engine is the bottleneck and the PE
            # has idle slack.
            fills = []

            def F(label, fn):
                fills.append((label, fn))

            for sec in (False, True):
                F("h0i1", lambda sec=sec: emit_lat_half("kv", 1, sec))
            F("h0i2", lambda: emit_v(0, 1, mps))
            F("h0i3", lambda: emit_v(0, 2, mps))
            F("h0i4", lambda: emit_kt(1, 0, mps))
            F("h0i4", lambda: emit_v(0, 3, mps))
            F("h0i5", lambda: emit_v(1, 0, mps))
            F("h0i6", lambda: emit_v(1, 1, mps))
            F("h0i7", lambda: emit_v(1, 2, mps))
            F("h0i7", lambda: emit_v(1, 3, mps))
            for c in (1, 2, 3):
                for p in (0, 1):
                    F(f"kq{c}", lambda p=p, c=c: emit_qt(p, c, mps))
                    F(f"kq{c}", lambda p=p, c=c: emit_kt(p, c, mps))
            for p in (2, 3):
                for kind in ("kv", "q0", "q1"):
                    for sec in (False, True):
                        F("j1lat", lambda p=p, kind=kind, sec=sec:
                          emit_lat_half(kind, p, sec))
            F("j1q0", lambda: emit_qt(2, 0, mps))
            F("j1q0", lambda: emit_qt(3, 0, mps))
            F("j1k8c0", lambda: emit_kt(2, 0, mps))
            for q in range(4):
                F("j1v2", lambda q=q: emit_v(2, q, mps))
            F("j1k12c0", lambda: emit_kt(3, 0, mps))
            for q in range(4):
                F("j1v3", lambda q=q: emit_v(3, q, mps))
            for c in (1, 2, 3):
                F(f"j1q{c}", lambda c=c: emit_qt(2, c, mps))
                F(f"j1q{c}", lambda c=c: emit_qt(3, c, mps))
                F(f"j1k8c{c}", lambda c=c: emit_kt(2, c, mps))
                F(f"j1k12c{c}", lambda c=c: emit_kt(3, c, mps))
            for si in range(8):
                F("op0", lambda si=si: emit_outproj_half(si, 0, mps))
                F("op0", lambda si=si: emit_outproj_half(si, 512, mps))

            fill_pos = [0]
            CAP = max(i for i, (lb, _) in enumerate(fills) if lb == "j1q0") + 1

            def drain(n, cap=None):
                lim = len(fills) if cap is None else cap
                while n > 0 and fill_pos[0] < lim:
                    fills[fill_pos[0]][1]()
                    fill_pos[0] += 1
                    n -= 1

            def drain_until(label):
                idx = max((i for i, (lb, _) in enumerate(fills) if lb == label),
                          default=-1)
                while fill_pos[0] <= idx:
                    fills[fill_pos[0]][1]()
                    fill_pos[0] += 1

            def need_j0(h):
                def need(k):
                    if h == 0 and 1 <= k <= 7:
                        drain_until(f"h0i{k}")
                    if k == 0:
                        if h == 1:
                            drain_until("h0i7")
                        elif h >= 2:
                            drain_until(f"kq{h // 2}")
                return need

            def need_j1(h):
                c = h // 2

                def need(k):
                    if k == 0:
                        drain_until(f"j1q{c}")
                    elif k == 8:
                        drain_until("j1v2" if c == 0 else f"j1k8c{c}")
                    elif k == 12:
                        drain_until("j1v3" if c == 0 else f"j1k12c{c}")
                return need

            segs = []
            for h in range(NHL):
                segs.append((0, h, lambda: drain(1, cap=CAP), need_j0(h)))
            for h in range(NHL):
                segs.append((1, h, lambda: drain(1), need_j1(h)))

            with (
                tc.tile_pool(name="scp", bufs=2, space="PSUM") as scp,
                tc.tile_pool(name="ctp", bufs=1, space="PSUM") as ctp,
            ):
                attn_stream(scp, ctp, segs)
                drain(len(fills))

            # ================= W4: out_proj j1 tail =========================
            for si in range(8, 16):
                emit_outproj_half(si, 0, mps)
                emit_outproj_half(si, 512, mps)

    nc.finalize()
    return nc


def shard_inputs(inputs, S=2048):
    """Build the 8 per-core input maps from full inputs (host-side prep:
    transpose x, cast matmul operands to bf16, pre-broadcast row biases)."""
    import ml_dtypes
    bf = lambda a: np.ascontiguousarray(np.asarray(a)).astype(ml_dtypes.bfloat16)
    f = lambda a: np.ascontiguousarray(np.asarray(a, dtype=np.float32))
    x = np.asarray(inputs["x"], dtype=np.float32)
    w_kvc, b_kvc = inputs["w_kvc"], f(inputs["b_kvc"])
    w_kvu, b_kvu = np.asarray(inputs["w_kvu"]), f(inputs["b_kvu"])
    w_qc, b_qc = inputs["w_qc"], f(inputs["b_qc"])
    w_qu, b_qu = np.asarray(inputs["w_qu"]), f(inputs["b_qu"])
    w_o, b_o = np.asarray(inputs["w_o"]), f(inputs["b_o"])
    xT = [bf(x[b].T) for b in range(B)]
    w_kvc_b = bf(w_kvc)
    w_qc_b = bf(w_qc)
    in_maps = []
    for core in range(NCORES):
        b = core // 2
        g = core % 2
        cs = slice(512 * g, 512 * g + 512)
        in_maps.append({
            "xT": xT[b],
            "w_kvc": w_kvc_b,
            "w_qc": w_qc_b,
            "w_kvu_k": bf(w_kvu[:, cs]),
            "w_kvu_v": bf(w_kvu[:, 1024 + 512 * g:1024 + 512 * g + 512]),
            "w_qu": bf(w_qu[:, cs]),
            "w_o": bf(w_o[cs, :]),
            "b_kvc": b_kvc.reshape(LAT, 1),
            "b_qc": np.ascontiguousarray(b_qc.reshape(2, 128).T),
            "b_qu": np.ascontiguousarray(b_qu[cs].reshape(4, 128).T),
            "b_kvu_k": np.ascontiguousarray(b_kvu[cs].reshape(4, 128).T),
            "b_kvu_v": np.ascontiguousarray(np.broadcast_to(
                b_kvu[1024 + 512 * g:1024 + 512 * g + 512], (128, 512))),
            "b_o": np.ascontiguousarray(np.broadcast_to(b_o * 0.5, (128, DIM))),
        })
    return in_maps


def kernel(**inputs) -> np.ndarray:
    from concourse.bass_utils import run_bass_kernel_spmd

    x = np.asarray(inputs["x"])
    S = x.shape[1]
    nc = build_mla(S=S)
    in_maps = shard_inputs(inputs, S=S)
    res = run_bass_kernel_spmd(nc, in_maps, list(range(NCORES))).results
    out = np.empty((B, S, DIM), dtype=np.float32)
    for b in range(B):
        out[b] = res[2 * b]["out"] + res[2 * b + 1]["out"]
    return out
